# revision 13
# baseline (speedup 1.0000x reference)
"""CenterLossLayer Trainium2 kernel — 8-core SPMD.

Math (reference):
    sel   = onehot @ centers                      # [B, D] — a row gather
    delta = onehot.T @ (sel - features)           # [C, D] — a scatter-add
    counts = onehot.sum(0) + 1                    # [C, 1]
    new_centers = centers - ALPHA * delta / counts
    loss = sum((features - sel)^2, axis=1)        # [B, 1]

Key identity: since row i of `onehot @ centers` is exactly centers[label_i],
    delta = counts ⊙ centers − onehot.T @ features
so the inter-core exchange only needs the LABELS (a few KB), not diffs:
    new_centers = centers·s1 + (onehot.T @ features)·s2,
    s1 = (1−ALPHA) + ALPHA/(counts+1),  s2 = ALPHA/(counts+1).

Structure (per core):
  Phase 1 (batch-sharded, 4 blocks of 128 rows): one DVE max_index over each
  f32 onehot block (row max is known to be 1.0) -> labels; indirect-DMA
  gather centers[labels]; loss = rowsum((sel-features)^2) in f32. AllGather
  the block's labels as [hi|lo] bf16 (4 tiny pipelined collectives).
  Phase 3 (class-sharded, 1250 classes): stage features (input) as bf16
  rhs chunks [feat|1]; per gathered 128-row chunk rebuild its one-hot tile
  bf16 — ScalarE Square(ciota−label) + DVE is_le(0.5) (exact for integers) —
  and accumulate onehot.T @ [feat|1] on PE into PSUM (ones column gives
  counts). Classes go in two 625-wide halves (5 PSUM banks each): half A
  pipelined with phase 1, half B as a second pass. Then the s1/s2 update.
Outputs per core: 512 loss rows + 1250 rows of new_centers; host concats.
"""
import os
import sys

import numpy as np

sys.path.insert(0, "/opt/trn_rl_repo")

import concourse.bass as bass  # noqa: E402
import concourse.tile as tile  # noqa: E402
from concourse import bacc, mybir  # noqa: E402
from concourse.bass import IndirectOffsetOnAxis  # noqa: E402
from concourse.bass_utils import run_bass_kernel_spmd  # noqa: E402

ALPHA = 0.5
B, C, D = 4096, 10000, 256
N_CORES = 8
BL = B // N_CORES          # 512 batch rows per core
CL = C // N_CORES          # 1250 classes per core
P = 128
NBLK = BL // P             # 4 local batch blocks
NGBLK = B // P             # 32 global 128-row chunks
EX = D + 1                 # rhs chunk row: feat | 1
HALF = 625                 # class half-shard (5 m-tiles of 125)
F32 = mybir.dt.float32
BF16 = mybir.dt.bfloat16
I32 = mybir.dt.int32
U32 = mybir.dt.uint32
AX = mybir.AxisListType
OP = mybir.AluOpType
AF = mybir.ActivationFunctionType

_CACHE = {}
STAGE = os.environ.get("KERNEL_STAGE", "full")


def _build():
    nc = bacc.Bacc("TRN2", target_bir_lowering=False, debug=False,
                   num_devices=N_CORES)
    features_l = nc.dram_tensor("features_l", [BL, D], F32,
                                kind="ExternalInput").ap()
    features_full = nc.dram_tensor("features_full", [B, D], F32,
                                   kind="ExternalInput").ap()
    onehot_l = nc.dram_tensor("onehot_l", [BL, C], F32,
                              kind="ExternalInput").ap()
    centers_full = nc.dram_tensor("centers_full", [C, D], F32,
                                  kind="ExternalInput").ap()
    centers_l = nc.dram_tensor("centers_l", [CL, D], F32,
                               kind="ExternalInput").ap()
    ciota_rep = nc.dram_tensor("ciota_rep", [P, CL], F32,
                               kind="ExternalInput").ap()
    loss_l = nc.dram_tensor("loss_l", [BL, 1], F32,
                            kind="ExternalOutput").ap()
    newc_l = nc.dram_tensor("newc_l", [CL, D], F32,
                            kind="ExternalOutput").ap()

    with tile.TileContext(nc) as tc:
        with tc.tile_pool(name="const", bufs=1) as constp, \
             tc.tile_pool(name="oh", bufs=2) as ohp, \
             tc.tile_pool(name="p1", bufs=2) as p1, \
             tc.tile_pool(name="p3", bufs=1) as p3, \
             tc.tile_pool(name="fst", bufs=2) as fstp, \
             tc.tile_pool(name="p3s", bufs=2) as p3s, \
             tc.tile_pool(name="recon", bufs=4) as reconp, \
             tc.tile_pool(name="sqs", bufs=4) as sqp, \
             tc.tile_pool(name="psum", bufs=8, space="PSUM") as psp, \
             tc.tile_pool(name="dram", bufs=1, space="DRAM") as dram:

            # ---- static constants ----
            ones8 = constp.tile([P, 8], F32, name="ones8")
            nc.vector.memset(ones8[:], 1.0)
            ciota_sb = constp.tile([P, CL], F32, name="ciota_sb")
            nc.sync.dma_start(ciota_sb[:], ciota_rep[:])

            ag_ins = [dram.tile([P, 2], BF16, name=f"ag_in{k}")
                      for k in range(NBLK)]
            ag_outs = [dram.tile([N_CORES * P, 2], BF16, addr_space="Shared",
                                 name=f"ag_out{k}") for k in range(NBLK)]

            # phase-3 SBUF-resident data
            rhs_all = p3.tile([P, NGBLK * EX], BF16, name="rhs_all")
            labs_all = p3.tile([P, NGBLK], F32, name="labs_all")
            neg_labs = p3.tile([P, NGBLK], F32, name="neg_labs")

            # ---- stage features -> bf16 rhs chunks [feat|1] ----
            for q in range(4):  # 8 chunks per DMA
                fst = fstp.tile([P, 8 * D], F32, tag="fst")
                src = features_full[q * 8 * P:(q + 1) * 8 * P, :]
                nc.sync.dma_start(
                    fst[:].rearrange("p (n d) -> p n d", n=8),
                    src.rearrange("(n p) d -> p n d", p=P))
                dst = rhs_all[:, q * 8 * EX:(q + 1) * 8 * EX]
                nc.vector.tensor_copy(
                    dst.rearrange("p (n e) -> p n e", n=8)[:, :, 0:D],
                    fst[:].rearrange("p (n d) -> p n d", n=8))
                nc.vector.memset(
                    dst.rearrange("p (n e) -> p n e", n=8)[:, :, D:EX], 1.0)

            # ---- phase 1: labels, loss, tiny label AllGather per block ----
            for k in range(NBLK):
                rows = slice(k * P, (k + 1) * P)
                oh = ohp.tile([P, C], F32, tag="oh")
                for c in range(4):
                    nc.sync.dma_start(
                        oh[:, c * (C // 4):(c + 1) * (C // 4)],
                        onehot_l[rows, c * (C // 4):(c + 1) * (C // 4)])
                idx8 = p1.tile([P, 8], U32, tag="idx8")
                nc.vector.max_index(idx8[:], ones8[:], oh[:])
                labels_i = p1.tile([P, 1], I32, tag="labels_i")
                nc.vector.tensor_copy(labels_i[:], idx8[:, 0:1])

                sel = p1.tile([P, D], F32, tag="sel")
                nc.gpsimd.indirect_dma_start(
                    out=sel[:], out_offset=None, in_=centers_full[:],
                    in_offset=IndirectOffsetOnAxis(ap=labels_i[:, :1], axis=0))
                feat = p1.tile([P, D], F32, tag="feat")
                nc.sync.dma_start(feat[:], features_l[rows, :])
                diff = p1.tile([P, D], F32, tag="diff")
                nc.vector.tensor_sub(diff[:], sel[:], feat[:])
                sq = p1.tile([P, D], F32, tag="sq")
                loss_t = p1.tile([P, 1], F32, tag="loss_t")
                nc.scalar.activation(out=sq[:], in_=diff[:], func=AF.Square)
                nc.vector.reduce_sum(loss_t[:], sq[:], axis=AX.X)
                nc.sync.dma_start(loss_l[rows, :], loss_t[:])

                exch = p1.tile([P, 2], BF16, tag="exch")
                hi_i = p1.tile([P, 1], I32, tag="hi_i")
                nc.vector.tensor_scalar(out=hi_i[:], in0=labels_i[:],
                                        scalar1=7, scalar2=None,
                                        op0=OP.arith_shift_right)
                lo_i = p1.tile([P, 1], I32, tag="lo_i")
                nc.vector.tensor_scalar(out=lo_i[:], in0=labels_i[:],
                                        scalar1=127, scalar2=None,
                                        op0=OP.bitwise_and)
                nc.vector.tensor_copy(exch[:, 0:1], hi_i[:])
                nc.vector.tensor_copy(exch[:, 1:2], lo_i[:])
                nc.sync.dma_start(ag_ins[k][:], exch[:])
                if STAGE != "p1":
                    nc.gpsimd.collective_compute(
                        "AllGather", OP.bypass,
                        replica_groups=[list(range(N_CORES))],
                        ins=[ag_ins[k].opt()], outs=[ag_outs[k].opt()])

            if STAGE == "p1":
                nc.compile()
                return nc

            # ---- labels of all 32 chunks from the 4 gathered buffers ----
            for k in range(NBLK):
                hl = p3s.tile([P, 8 * 2], BF16, tag="hl")
                nc.sync.dma_start(
                    hl[:].rearrange("p (n e) -> p n e", n=8),
                    ag_outs[k][:].rearrange("(n p) e -> p n e", p=P))
                # core j's block-k labels belong to global rows 512j+128k,
                # i.e. features chunk 4j+k -> strided positions k, k+4, ...
                t_hi = p3s.tile([P, 8], F32, tag="t_hi")
                nc.vector.tensor_scalar(
                    out=t_hi[:],
                    in0=hl[:].rearrange("p (n e) -> p n e", n=8)[:, :, 0:1],
                    scalar1=128.0, scalar2=None, op0=OP.mult)
                nc.vector.tensor_tensor(
                    out=labs_all[:, k:NGBLK:NBLK], in0=t_hi[:],
                    in1=hl[:].rearrange("p (n e) -> p n e", n=8)[:, :, 1:2],
                    op=OP.add)
                nc.vector.tensor_scalar(out=neg_labs[:, k:NGBLK:NBLK],
                                        in0=labs_all[:, k:NGBLK:NBLK],
                                        scalar1=-1.0, scalar2=None,
                                        op0=OP.mult)

            # ---- phase 3: recon + matmul, two class halves ----
            mts = [(m0, min(P, HALF - m0)) for m0 in range(0, HALF, P)]

            def half_pass(c0, psums):
                for g in range(NGBLK):
                    sqt = sqp.tile([P, HALF], F32, tag="sq_scr",
                                   name=f"sq_{c0}_{g}")
                    nc.scalar.activation(out=sqt[:],
                                         in_=ciota_sb[:, c0:c0 + HALF],
                                         func=AF.Square,
                                         bias=neg_labs[:, g:g + 1], scale=1.0)
                    recon = reconp.tile([P, HALF], BF16, tag="recon",
                                        name=f"recon_{c0}_{g}")
                    nc.vector.tensor_scalar(out=recon[:], in0=sqt[:],
                                            scalar1=0.5, scalar2=None,
                                            op0=OP.is_le)
                    for i, (m0, msz) in enumerate(mts):
                        nc.tensor.matmul(
                            out=psums[i][:msz, :],
                            lhsT=recon[:, m0:m0 + msz],
                            rhs=rhs_all[:, g * EX:(g + 1) * EX],
                            start=(g == 0), stop=(g == NGBLK - 1))

            psA = [psp.tile([P, EX], F32, tag="delta_ps", name=f"psA_{i}")
                   for i in range(len(mts))]
            half_pass(0, psA)
            psB = [psp.tile([P, EX], F32, tag="delta_ps", name=f"psB_{i}")
                   for i in range(len(mts))]
            half_pass(HALF, psB)

            # ---- update: newc = centers*s1 + mm*s2 ----
            for c0, psums in ((0, psA), (HALF, psB)):
                for i, (m0, msz) in enumerate(mts):
                    mabs = c0 + m0
                    cnt1 = p3s.tile([P, 1], F32, tag="cnt1")
                    nc.vector.tensor_scalar_add(
                        cnt1[:msz], psums[i][:msz, D:D + 1], 1.0)
                    recip = p3s.tile([P, 1], F32, tag="recip")
                    nc.vector.reciprocal(recip[:msz], cnt1[:msz])
                    s2 = p3s.tile([P, 1], F32, tag="s2")
                    nc.vector.tensor_scalar_mul(s2[:msz], recip[:msz], ALPHA)
                    s1 = p3s.tile([P, 1], F32, tag="s1")
                    nc.vector.tensor_scalar(out=s1[:msz], in0=recip[:msz],
                                            scalar1=ALPHA,
                                            scalar2=1.0 - ALPHA,
                                            op0=OP.mult, op1=OP.add)
                    cen = p3s.tile([P, D], F32, tag="cen")
                    nc.sync.dma_start(cen[:msz], centers_l[mabs:mabs + msz, :])
                    t1 = p3s.tile([P, D], F32, tag="t1")
                    nc.scalar.activation(out=t1[:msz], in_=cen[:msz],
                                         func=AF.Copy, scale=s1[:msz, :1])
                    t2 = p3s.tile([P, D], F32, tag="t2")
                    nc.vector.tensor_scalar(out=t2[:msz],
                                            in0=psums[i][:msz, 0:D],
                                            scalar1=s2[:msz, :1],
                                            scalar2=None, op0=OP.mult)
                    newc = p3s.tile([P, D], F32, tag="newc")
                    nc.vector.tensor_add(newc[:msz], t1[:msz], t2[:msz])
                    nc.sync.dma_start(newc_l[mabs:mabs + msz, :], newc[:msz])
    nc.compile()
    return nc


def _get_nc():
    if "nc" not in _CACHE:
        _CACHE["nc"] = _build()
    return _CACHE["nc"]


def _in_maps(features, onehot, centers):
    maps = []
    for i in range(N_CORES):
        ciota = np.broadcast_to(
            np.arange(i * CL, (i + 1) * CL, dtype=np.float32)[None, :],
            (P, CL)).copy()
        maps.append({
            "features_l": features[i * BL:(i + 1) * BL],
            "features_full": features,
            "onehot_l": onehot[i * BL:(i + 1) * BL],
            "centers_full": centers,
            "centers_l": centers[i * CL:(i + 1) * CL],
            "ciota_rep": ciota,
        })
    return maps


def kernel(features, onehot, centers):
    features = np.ascontiguousarray(features, dtype=np.float32)
    onehot = np.ascontiguousarray(onehot, dtype=np.float32)
    centers = np.ascontiguousarray(centers, dtype=np.float32)
    nc = _get_nc()
    res = run_bass_kernel_spmd(nc, _in_maps(features, onehot, centers),
                               core_ids=list(range(N_CORES)))
    loss = np.concatenate([res.results[i]["loss_l"] for i in range(N_CORES)],
                          axis=0)
    new_centers = np.concatenate(
        [res.results[i]["newc_l"] for i in range(N_CORES)], axis=0)
    return loss, new_centers
